# revision 43
# baseline (speedup 1.0000x reference)
"""Distributed attention kernel for 8 TRN2 NeuronCores.

Sharding: tensor-parallel over heads (2 heads/core, Megatron column split of
w_qkv), attention computed per-core for its heads over all batches, then a
per-batch AllToAll redistributes the (transposed) attention output so each
core runs the output projection for 1/8 of the tokens (256 per batch) against
the full w_proj.

Layout: everything is kept transposed (d on partitions) so that
  - scores come out as S^T (keys on partitions, queries on free axis),
  - softmax needs no max subtraction (logits ~ N(0,1)),
  - the two heads run as row/col-tiled concurrent matmul pairs using the full
    128-wide PE array,
  - the projection consumes the transposed attention output directly.
Compute dtype is bf16 with f32 PSUM accumulation.

The build is software-pipelined: QKV for batch b+1 and the projection for
batch b-1 are interleaved into attention(b)'s inner loop as filler units so
the TensorEngine never idles long enough for the HAM clock gate to throttle
it. Softmax denominators are accumulated on the VectorEngine (two bf16
accumulators per strip), reduced across partitions by a ones-matmul, batched
through DRAM so one DVE reciprocal serves a whole batch, and broadcast back
by a partition-stride-0 DMA.

Perf additions over the first working version:
  - exp table preloaded + a tiny warmup AllToAll at t=0 so neither the
    ~2.7us ACT table load nor the ~30us first-collective warmup lands on
    the critical path,
  - strip-end finalize work (attnT copy, denominator matmuls) deferred into
    the next strip's filler slots so the in-order Vector/PE queues never
    stall the ScalarE exp cadence at strip boundaries,
  - approximate-reciprocal custom DVE op (~5x faster),
  - the last batch redistributes + projects in two halves overlapped with
    its own attention, shrinking the serial tail.
"""

import os
import sys

import numpy as np

for _p in ("/opt/trn_rl_repo", os.path.expanduser("~/.axon_site/_ro/trn_rl_repo")):
    if os.path.isdir(_p) and _p not in sys.path:
        sys.path.insert(0, _p)

import ml_dtypes  # noqa: E402

import concourse.bass as bass  # noqa: E402
from concourse import bacc, bass_isa, mybir  # noqa: E402
import concourse.tile as tile  # noqa: E402
from concourse.bass_utils import run_bass_kernel_spmd  # noqa: E402

B, N, DIM, H = 4, 2048, 1024, 16
HD = DIM // H            # 64 head dim
NCORES = 8
HPC = H // NCORES        # 2 heads per core
HC = HPC * HD            # 128 head-cols per core
T = B * N                # 8192 tokens
QT_TOK = N // NCORES     # 256 tokens per core per batch
SCALE = HD ** -0.5

BF16 = mybir.dt.bfloat16
F32 = mybir.dt.float32
EXP = mybir.ActivationFunctionType.Exp

LAST_RESULTS = None  # BassKernelResults of the most recent run (for test.py)


def _build():
    nc = bacc.Bacc(num_devices=NCORES)

    x_t = nc.declare_dram_parameter("x_t", [DIM, T], BF16, isOutput=False)
    w_c = nc.declare_dram_parameter("w_c", [DIM, 3 * HC], BF16, isOutput=False)
    w_p = nc.declare_dram_parameter("w_p", [DIM, DIM], BF16, isOutput=False)
    b_p = nc.declare_dram_parameter("b_p", [DIM], F32, isOutput=False)
    sel_c = nc.declare_dram_parameter("sel_c", [2, 2 * 64], F32, isOutput=False)
    # columns: batch-major quarters of 256 tokens (b3: 128-token eighths)
    out_t = nc.declare_dram_parameter("out_t", [DIM, B * QT_TOK], F32, isOutput=True)

    with tile.TileContext(nc) as tc:
        with (
            tc.tile_pool(name="persist", bufs=1) as persist,
            tc.tile_pool(name="xin", bufs=4) as xin,
            tc.tile_pool(name="work", bufs=3) as work,
            tc.tile_pool(name="ps_mm", bufs=2, space="PSUM") as ps_mm,
            tc.tile_pool(name="ps_s", bufs=2, space="PSUM") as ps_s,
            tc.tile_pool(name="ps_o", bufs=2, space="PSUM") as ps_o,
            tc.tile_pool(name="dram", bufs=1, space="DRAM") as dram,
        ):
            # ---- persistent SBUF tensors ----
            wqkv_sb = persist.tile([128, 8, 3 * HC], BF16)
            wproj_sb = persist.tile([128, 8, DIM], BF16)
            bias_sb = persist.tile([128, 8], F32)
            ones_sb = persist.tile([128, 1], BF16)
            QT = persist.tile([128, T], BF16)
            KT = persist.tile([128, T], BF16)
            Vp = persist.tile([128, B, 16, HPC, HD], BF16)
            attnT = persist.tile([128, T], BF16)   # rows = h*64 + d
            warm_sb = persist.tile([1, 1], F32)
            eye2 = persist.tile([128, 2, 2], BF16)   # one-hot cols per head
            sel = persist.tile([2, 2, 64], F32)      # row-select for bcast mm

            # ---- DRAM staging ----
            ag_in = dram.tile([B, NCORES * HC, QT_TOK], BF16)
            ag_out = dram.tile([B, NCORES * HC, QT_TOK], BF16)
            ag_in3 = dram.tile([2, NCORES * HC, 128], BF16)
            ag_out3 = dram.tile([2, NCORES * HC, 128], BF16)
            wa_in = dram.tile([NCORES, 16], BF16)
            wa_out = dram.tile([NCORES, 16], BF16)

            # warm up the ACT exp table and the collectives path immediately
            # so neither first-use cost lands mid-kernel
            nc.vector.memset(ones_sb, 1.0)
            nc.vector.memset(eye2, 0.0)
            nc.vector.memset(eye2[:, 0, 0:1], 1.0)
            nc.vector.memset(eye2[:, 1, 1:2], 1.0)
            nc.sync.dma_start(sel, sel_c[:, :])
            nc.scalar.activation(warm_sb, ones_sb[0:1, 0:1], EXP, scale=1.0)
            nc.gpsimd.collective_compute(
                "AllToAll", mybir.AluOpType.bypass,
                replica_groups=[list(range(NCORES))],
                ins=[wa_in], outs=[wa_out],
            )

            for k in range(8):
                nc.sync.dma_start(wqkv_sb[:, k, :], w_c[k * 128:(k + 1) * 128, :])

            # ---- phase builders ----
            def qkv_chunk_units(tq, kv_first=False):
                """1024-token QKV chunk as a list of filler closures."""
                st = {}

                def u_dma():
                    xt = xin.tile([128, 8, 1024], BF16, tag="xt", name=f"xt{tq}")
                    c0 = tq * 1024
                    for k in range(8):
                        nc.sync.dma_start(
                            xt[:, k, :],
                            x_t[k * 128:(k + 1) * 128, c0:c0 + 1024],
                        )
                    st["xt"] = xt

                def mk_qk(m, nh, half):
                    def u():
                        xt = st["xt"]
                        if half == 0:
                            st[(m, nh)] = ps_mm.tile(
                                [128, 512], F32, tag="mm", name=f"pqk{tq}{m}{nh}"
                            )
                        pmm = st[(m, nh)]
                        for k in range(4 * half, 4 * half + 4):
                            nc.tensor.matmul(
                                pmm,
                                wqkv_sb[:, k, m * 128:(m + 1) * 128],
                                xt[:, k, nh * 512:(nh + 1) * 512],
                                start=(k == 0),
                                stop=(k == 7),
                            )
                        if half == 1:
                            dst = QT if m == 0 else KT
                            nc.vector.tensor_copy(
                                dst[:, tq * 1024 + nh * 512:
                                    tq * 1024 + (nh + 1) * 512],
                                pmm,
                            )
                    return u

                def mk_v(st_idx):
                    def u():
                        xt = st["xt"]
                        pv = ps_mm.tile([128, 128], F32, tag="mm", name=f"pv{tq}{st_idx}")
                        for k in range(8):
                            nc.tensor.matmul(
                                pv,
                                xt[:, k, st_idx * 128:(st_idx + 1) * 128],
                                wqkv_sb[:, k, 2 * HC:3 * HC],
                                start=(k == 0),
                                stop=(k == 7),
                            )
                        gt = tq * 8 + st_idx
                        b, lt = gt // 16, gt % 16
                        nc.vector.tensor_copy(Vp[:, b, lt, :, :], pv)
                    return u

                if kv_first:
                    # K and V as early as possible (attention consumes them
                    # incrementally along the kj axis); Q strips last
                    units = [u_dma]
                    for nh in range(2):
                        units += [mk_qk(1, nh, 0), mk_qk(1, nh, 1)]
                    for st_idx in range(8):
                        units.append(mk_v(st_idx))
                    for nh in range(2):
                        units += [mk_qk(0, nh, 0), mk_qk(0, nh, 1)]
                else:
                    units = [u_dma]
                    for m in range(2):
                        for nh in range(2):
                            units.append(mk_qk(m, nh, 0))
                            units.append(mk_qk(m, nh, 1))
                    for st_idx in range(8):
                        units.append(mk_v(st_idx))
                return units

            def proj_quarter_units(b):
                """Projection of this core's 256 batch-b tokens as fillers."""
                st = {}

                def u_dma():
                    # gpsimd queue: a not-yet-finished AllToAll must not block
                    # the sync queue's x/staging traffic
                    agT = work.tile([128, 8, QT_TOK], BF16, tag="agT", name=f"agT{b}")
                    for r in range(NCORES):
                        nc.gpsimd.dma_start(
                            agT[:, r, :], ag_out[b, r * HC:(r + 1) * HC, :]
                        )
                    st["agT"] = agT

                def mk_od(od, half):
                    def u():
                        agT = st["agT"]
                        if half == 0:
                            st[od] = ps_mm.tile(
                                [128, QT_TOK], F32, tag="mm", name=f"pp{b}{od}"
                            )
                        pp = st[od]
                        for r in range(4 * half, 4 * half + 4):
                            nc.tensor.matmul(
                                pp,
                                wproj_sb[:, r, od * 128:(od + 1) * 128],
                                agT[:, r, :],
                                start=(r == 0),
                                stop=(r == 7),
                            )
                        if half == 1:
                            ob = work.tile([128, QT_TOK], F32, tag="ob",
                                           name=f"ob{b}{od}")
                            nc.vector.tensor_scalar_add(
                                ob, pp, bias_sb[:, od:od + 1]
                            )
                            nc.sync.dma_start(
                                out_t[od * 128:(od + 1) * 128,
                                      b * QT_TOK:(b + 1) * QT_TOK],
                                ob,
                            )
                    return u

                return [u_dma] + [mk_od(od, hf) for od in range(8) for hf in (0, 1)]

            def a2a3_stage(half, lo, hi):
                """Stage token chunks lo..hi of the last batch's half."""
                base = 3 * N + half * 1024
                for j in range(lo, hi):
                    nc.sync.dma_start(
                        ag_in3[half, j * HC:(j + 1) * HC, :],
                        attnT[:, base + j * 128:base + (j + 1) * 128],
                    )

            def a2a3_go(half):
                nc.gpsimd.collective_compute(
                    "AllToAll", mybir.AluOpType.bypass,
                    replica_groups=[list(range(NCORES))],
                    ins=[ag_in3[half]], outs=[ag_out3[half]],
                )

            def proj_half_units(half):
                st = {}

                def u_dma():
                    agT = work.tile([128, 8, 128], BF16, tag="agT3", bufs=2,
                                    name=f"agT3{half}")
                    for r in range(NCORES):
                        # half 1 runs after attention: ScalarE is idle, split
                        # the loads across two queues to halve issue latency
                        eng = nc.scalar if (half == 1 and r % 2) else nc.gpsimd
                        eng.dma_start(
                            agT[:, r, :], ag_out3[half, r * HC:(r + 1) * HC, :]
                        )
                    st["agT"] = agT

                def mk_od(od):
                    def u():
                        agT = st["agT"]
                        pp = ps_mm.tile([128, 128], F32, tag="mm",
                                        name=f"p3{half}{od}")
                        for r in range(8):
                            nc.tensor.matmul(
                                pp, wproj_sb[:, r, od * 128:(od + 1) * 128],
                                agT[:, r, :], start=(r == 0), stop=(r == 7),
                            )
                        ob = work.tile([128, 128], F32, tag="ob",
                                       name=f"ob3{half}{od}")
                        nc.vector.tensor_scalar_add(ob, pp, bias_sb[:, od:od + 1])
                        nc.sync.dma_start(
                            out_t[od * 128:(od + 1) * 128,
                                  3 * QT_TOK + half * 128:
                                  3 * QT_TOK + half * 128 + 128],
                            ob,
                        )
                    return u

                return [u_dma] + [mk_od(od) for od in range(8)]

            # ---- minimal batch-0 prologue: x chunk 0, Q/K for kj 0-3, V 0-1
            ch0 = qkv_chunk_units(0)
            for i in (0, 1, 2, 5, 6, 9, 10):
                ch0[i]()
            ch0_rest = [ch0[i] for i in (7, 8, 11, 12, 13, 14, 15, 16, 3, 4)]
            for k in range(8):
                nc.sync.dma_start(wproj_sb[:, k, :], w_p[k * 128:(k + 1) * 128, :])
                nc.sync.dma_start(bias_sb[:, k:k + 1], b_p[k * 128:(k + 1) * 128])

            # ---- pipelined main loop over batches ----
            for b in range(B):
                t0 = b * N
                if b == 0:
                    ch1 = qkv_chunk_units(1, kv_first=True)
                    ch2 = qkv_chunk_units(2)
                    ch3 = qkv_chunk_units(3)
                    # front-load all of batch 0's x DMAs: the ramp is
                    # bandwidth-bound, keep every DMA queue streaming
                    fillers = ([ch1[0], ch2[0], ch3[0]] + ch0_rest
                               + ch1[1:] + ch2[1:] + ch3[1:])
                else:
                    fillers = []
                    if b + 1 < B:
                        fillers += qkv_chunk_units(2 * (b + 1))
                        fillers += qkv_chunk_units(2 * (b + 1) + 1)
                    fillers += proj_quarter_units(b - 1)
                # qkv fillers are paced from iteration 0; proj fillers (which
                # wait on the previous batch's AllToAll) only from PROJ_AT on.
                n_qkv = len(fillers) - (17 if b >= 1 else 0)
                PROJ_AT = 26
                fillers.reverse()  # pop() from the end = original order
                n_fill = len(fillers)
                popped = 0

                def push_next(units):
                    """Insert units so they are popped before the backlog."""
                    nonlocal n_fill
                    fillers.extend(reversed(units))
                    n_fill += len(units)

                fin_q = []

                def pop_fin():
                    if fin_q:
                        fin_q.pop(0)()

                def finalize_units(qi, po, acc, q0):
                    """Copy out numerators, then normalize the strip without
                    touching DRAM: ones-matmul partition-reduce -> approx
                    reciprocal straight off PSUM -> K=2 matmul broadcast ->
                    one DVE multiply -> stage for the AllToAll."""
                    st = {}

                    def u_copy():
                        nc.vector.tensor_copy(attnT[:, q0:q0 + 512], po)

                    def u_merge():
                        nc.vector.tensor_add(acc[0], acc[0], acc[1])

                    def u_den():
                        pden = ps_mm.tile([2, 512], F32, tag="mm",
                                          name=f"pden{b}{qi}")
                        for h in range(HPC):
                            nc.tensor.matmul(pden, eye2[:, h, :],
                                             acc[0][:, h, :],
                                             start=(h == 0), stop=(h == 1))
                        rdenf = work.tile([2, 512], F32, tag="rdenf",
                                          name=f"rdf{b}{qi}")
                        nc.vector.reciprocal_approx_fast(rdenf, pden)
                        st["rdenf"] = rdenf

                    def u_bc():
                        rdenf = st["rdenf"]
                        bc = ps_mm.tile([128, 512], F32, tag="mm",
                                        name=f"bc{b}{qi}")
                        for h in range(HPC):
                            nc.tensor.matmul(bc[64 * h:64 * (h + 1), :],
                                             sel[:, h, :], rdenf,
                                             start=True, stop=True)
                        nc.vector.tensor_mul(
                            attnT[:, q0:q0 + 512], attnT[:, q0:q0 + 512], bc
                        )
                        if b < B - 1:
                            # this strip's two AllToAll chunks go out now, so
                            # the batch-end trigger has nothing left to wait on
                            for j in (2 * qi, 2 * qi + 1):
                                nc.sync.dma_start(
                                    ag_in[b, j * HC:(j + 1) * HC, :],
                                    attnT[:, t0 + j * QT_TOK:
                                          t0 + (j + 1) * QT_TOK],
                                )

                    return [u_copy, u_merge, u_den, u_bc]

                # flat list of (qi, kj) steps; S^T pairs are emitted one step
                # ahead so the ACT-feeding matmul is never queued behind the
                # eS-gated V matmuls or filler work on the in-order PE queue
                steps = [(qi, kj) for qi in range(4) for kj in range(16)]
                pS_t = {}
                po_t = {}
                acc_t = {}

                def emit_S(qi, kj):
                    q0 = t0 + qi * 512
                    k0 = t0 + kj * 128
                    pS = ps_s.tile([128, 2, 512], F32, tag="s",
                                   name=f"pS{b}_{qi}_{kj}")
                    for h in range(HPC):
                        hs = h * HD
                        nc.tensor.matmul(
                            pS[:, h, :],
                            KT[hs:hs + HD, k0:k0 + 128],
                            QT[hs:hs + HD, q0:q0 + 512],
                            start=True,
                            stop=True,
                        )
                    pS_t[(qi, kj)] = pS

                emit_S(0, 0)
                for it, (qi, kj) in enumerate(steps):
                    q0 = t0 + qi * 512
                    if kj == 0:
                        po_t[qi] = ps_o.tile([128, 512], F32, tag="vo",
                                             name=f"po{b}_{qi}")
                        acc_t[qi] = [
                            work.tile([128, 2, 512], BF16, tag=f"acc{a}",
                                      name=f"acc{a}_{b}_{qi}")
                            for a in range(2)
                        ]
                    if b == B - 1:
                        # pipeline the last batch's own redistribute+project:
                        # half 0 overlaps strips 2-3, only strip 3's share of
                        # half 1 is left serial at the very end
                        if it == 36:
                            a2a3_stage(0, 0, 8)
                            a2a3_go(0)
                        elif it == 48:
                            push_next(proj_half_units(0))
                        elif it == 56:
                            a2a3_stage(1, 0, 4)
                    if b == 0 and it < 16:
                        target = 3 * (it + 1)
                    else:
                        target = (it + 1) * n_fill // 56
                    cap = 8 if (b == 0 and it < 16) else 4
                    quota = popped < n_qkv or it >= PROJ_AT
                    due = min(cap, max(0, target - popped)) if quota else 0
                    # sandwich the filler work around the two sem-gated
                    # instructions (S waiting its PSUM slot, V waiting eS) so
                    # the in-order PE queue never idles at a blocked head
                    pop_fin()
                    for _ in range(due // 2):
                        if fillers:
                            fillers.pop()()
                            popped += 1
                    if it + 1 < len(steps):
                        emit_S(*steps[it + 1])
                    pS = pS_t.pop((qi, kj))
                    eS = work.tile([128, 2, 512], BF16, tag="es", bufs=6)
                    nc.scalar.activation(eS, pS, EXP, scale=SCALE)
                    pop_fin()
                    for _ in range(due - due // 2):
                        if fillers:
                            fillers.pop()()
                            popped += 1
                    po, acc = po_t[qi], acc_t[qi]
                    for h in range(HPC):
                        nc.tensor.matmul(
                            po[h * HD:(h + 1) * HD, :],
                            Vp[:, b, kj, h, :],
                            eS[:, h, :],
                            start=(kj == 0),
                            stop=(kj == 15),
                        )
                    a = kj // 8
                    if kj % 8 == 0:
                        nc.vector.tensor_copy(acc[a], eS)
                    else:
                        nc.vector.tensor_add(acc[a], acc[a], eS)
                    if kj == 15:
                        fin_q += finalize_units(qi, po, acc, q0)
                while fin_q:
                    fin_q.pop(0)()
                while fillers:
                    fillers.pop()()

                if b < B - 1:
                    # ---- AllToAll for batch b (chunks staged per strip) ----
                    nc.gpsimd.collective_compute(
                        "AllToAll",
                        mybir.AluOpType.bypass,
                        replica_groups=[list(range(NCORES))],
                        ins=[ag_in[b]],
                        outs=[ag_out[b]],
                    )
                else:
                    # ---- tail: strip 3's share of the last batch ----
                    a2a3_stage(1, 4, 8)
                    a2a3_go(1)
                    for u in proj_half_units(1):
                        u()

    nc.finalize()
    return nc


def kernel(x, w_qkv, w_proj, b_proj):
    global LAST_RESULTS
    bf16 = ml_dtypes.bfloat16

    x_t = np.ascontiguousarray(x.reshape(T, DIM).T.astype(bf16))  # [DIM, T]
    w_p = np.ascontiguousarray(w_proj.astype(bf16))
    b_p = np.ascontiguousarray(b_proj.astype(np.float32))
    sel_np = np.zeros((2, 128), dtype=np.float32)
    sel_np[0, 0:64] = 1.0
    sel_np[1, 64:128] = 1.0

    in_maps = []
    for c in range(NCORES):
        w_c = np.concatenate(
            [
                w_qkv[:, HC * c:HC * (c + 1)],
                w_qkv[:, DIM + HC * c:DIM + HC * (c + 1)],
                w_qkv[:, 2 * DIM + HC * c:2 * DIM + HC * (c + 1)],
            ],
            axis=1,
        ).astype(bf16)
        in_maps.append(
            {"x_t": x_t, "w_c": np.ascontiguousarray(w_c), "w_p": w_p,
             "b_p": b_p, "sel_c": sel_np}
        )

    nc = _build()
    LAST_RESULTS = run_bass_kernel_spmd(
        nc, in_maps, core_ids=list(range(NCORES)),
        trace=bool(os.environ.get("KERNEL_TRACE")),
    )

    # core c's out_t columns: batches 0-2 are 256-token quarters; batch 3 is
    # two 128-token slices (tokens 128c and 1024+128c of the batch)
    out_T = np.empty((DIM, T), dtype=np.float32)
    for c in range(NCORES):
        res = np.asarray(LAST_RESULTS.results[c]["out_t"], dtype=np.float32)
        for b in range(B - 1):
            out_T[:, b * N + c * QT_TOK:b * N + (c + 1) * QT_TOK] = (
                res[:, b * QT_TOK:(b + 1) * QT_TOK]
            )
        b3 = (B - 1) * N
        for half in range(2):
            cols = 3 * QT_TOK + half * 128
            tok = b3 + half * 1024 + c * 128
            out_T[:, tok:tok + 128] = res[:, cols:cols + 128]
    return np.ascontiguousarray(out_T.T).reshape(B, N, DIM).astype(np.float32)


# revision 44
# speedup vs baseline: 1.0734x; 1.0734x over previous
"""Distributed attention kernel for 8 TRN2 NeuronCores.

Sharding: tensor-parallel over heads (2 heads/core, Megatron column split of
w_qkv), attention computed per-core for its heads over all batches, then a
per-batch AllToAll redistributes the (transposed) attention output so each
core runs the output projection for 1/8 of the tokens (256 per batch) against
the full w_proj.

Layout: everything is kept transposed (d on partitions) so that
  - scores come out as S^T (keys on partitions, queries on free axis),
  - softmax needs no max subtraction (logits ~ N(0,1)),
  - the two heads run as row/col-tiled concurrent matmul pairs using the full
    128-wide PE array,
  - the projection consumes the transposed attention output directly.
Compute dtype is bf16 with f32 PSUM accumulation.

The build is software-pipelined: QKV for batch b+1 and the projection for
batch b-1 are interleaved into attention(b)'s inner loop as filler units so
the TensorEngine never idles long enough for the HAM clock gate to throttle
it. Softmax denominators are accumulated on the VectorEngine (two bf16
accumulators per strip), reduced across partitions by a ones-matmul, batched
through DRAM so one DVE reciprocal serves a whole batch, and broadcast back
by a partition-stride-0 DMA.

Perf additions over the first working version:
  - exp table preloaded + a tiny warmup AllToAll at t=0 so neither the
    ~2.7us ACT table load nor the ~30us first-collective warmup lands on
    the critical path,
  - strip-end finalize work (attnT copy, denominator matmuls) deferred into
    the next strip's filler slots so the in-order Vector/PE queues never
    stall the ScalarE exp cadence at strip boundaries,
  - approximate-reciprocal custom DVE op (~5x faster),
  - the last batch redistributes + projects in two halves overlapped with
    its own attention, shrinking the serial tail.
"""

import os
import sys

import numpy as np

for _p in ("/opt/trn_rl_repo", os.path.expanduser("~/.axon_site/_ro/trn_rl_repo")):
    if os.path.isdir(_p) and _p not in sys.path:
        sys.path.insert(0, _p)

import ml_dtypes  # noqa: E402

import concourse.bass as bass  # noqa: E402
from concourse import bacc, bass_isa, mybir  # noqa: E402
import concourse.tile as tile  # noqa: E402
from concourse.bass_utils import run_bass_kernel_spmd  # noqa: E402

B, N, DIM, H = 4, 2048, 1024, 16
HD = DIM // H            # 64 head dim
NCORES = 8
HPC = H // NCORES        # 2 heads per core
HC = HPC * HD            # 128 head-cols per core
T = B * N                # 8192 tokens
QT_TOK = N // NCORES     # 256 tokens per core per batch
SCALE = HD ** -0.5

BF16 = mybir.dt.bfloat16
F32 = mybir.dt.float32
EXP = mybir.ActivationFunctionType.Exp

LAST_RESULTS = None  # BassKernelResults of the most recent run (for test.py)


def _build():
    nc = bacc.Bacc(num_devices=NCORES)

    x_t = nc.declare_dram_parameter("x_t", [DIM, T], BF16, isOutput=False)
    w_c = nc.declare_dram_parameter("w_c", [DIM, 3 * HC], BF16, isOutput=False)
    w_p = nc.declare_dram_parameter("w_p", [DIM, DIM], BF16, isOutput=False)
    b_p = nc.declare_dram_parameter("b_p", [DIM], F32, isOutput=False)
    sel_c = nc.declare_dram_parameter("sel_c", [2, 2 * 64], F32, isOutput=False)
    # columns: batch-major quarters of 256 tokens (b3: 128-token eighths)
    out_t = nc.declare_dram_parameter("out_t", [DIM, B * QT_TOK], F32, isOutput=True)

    with tile.TileContext(nc) as tc:
        with (
            tc.tile_pool(name="persist", bufs=1) as persist,
            tc.tile_pool(name="xin", bufs=3) as xin,
            tc.tile_pool(name="work", bufs=3) as work,
            tc.tile_pool(name="ps_mm", bufs=2, space="PSUM") as ps_mm,
            tc.tile_pool(name="ps_s", bufs=2, space="PSUM") as ps_s,
            tc.tile_pool(name="ps_o", bufs=2, space="PSUM") as ps_o,
            tc.tile_pool(name="dram", bufs=1, space="DRAM") as dram,
        ):
            # ---- persistent SBUF tensors ----
            wqkv_sb = persist.tile([128, 8, 3 * HC], BF16)
            wproj_sb = persist.tile([128, 8, DIM], BF16)
            bias_sb = persist.tile([128, 8], F32)
            ones_sb = persist.tile([128, 1], BF16)
            QT = persist.tile([128, T], BF16)
            KT = persist.tile([128, T], BF16)
            Vp = persist.tile([128, B, 16, HPC, HD], BF16)
            attnT = persist.tile([128, T], BF16)   # rows = h*64 + d
            warm_sb = persist.tile([1, 1], F32)
            eye2 = persist.tile([128, 2, 2], BF16)   # one-hot cols per head
            sel = persist.tile([2, 2, 64], F32)      # row-select for bcast mm

            # ---- DRAM staging ----
            ag_in = dram.tile([B, NCORES * HC, QT_TOK], BF16)
            ag_out = dram.tile([B, NCORES * HC, QT_TOK], BF16)
            ag_in3 = dram.tile([2, NCORES * HC, 128], BF16)
            ag_out3 = dram.tile([2, NCORES * HC, 128], BF16)
            wa_in = dram.tile([NCORES, 16], BF16)
            wa_out = dram.tile([NCORES, 16], BF16)

            # warm up the ACT exp table and the collectives path immediately
            # so neither first-use cost lands mid-kernel
            nc.vector.memset(ones_sb, 1.0)
            nc.vector.memset(eye2, 0.0)
            nc.vector.memset(eye2[:, 0, 0:1], 1.0)
            nc.vector.memset(eye2[:, 1, 1:2], 1.0)
            nc.sync.dma_start(sel, sel_c[:, :])
            nc.scalar.activation(warm_sb, ones_sb[0:1, 0:1], EXP, scale=1.0)
            nc.gpsimd.collective_compute(
                "AllToAll", mybir.AluOpType.bypass,
                replica_groups=[list(range(NCORES))],
                ins=[wa_in], outs=[wa_out],
            )

            for k in range(8):
                nc.sync.dma_start(wqkv_sb[:, k, :], w_c[k * 128:(k + 1) * 128, :])

            # ---- phase builders ----
            def qkv_chunk_units(tq, kv_first=False):
                """1024-token QKV chunk as a list of filler closures."""
                st = {}

                def u_dma():
                    xt = xin.tile([128, 8, 1024], BF16, tag="xt", name=f"xt{tq}")
                    c0 = tq * 1024
                    for k in range(8):
                        nc.sync.dma_start(
                            xt[:, k, :],
                            x_t[k * 128:(k + 1) * 128, c0:c0 + 1024],
                        )
                    st["xt"] = xt

                def mk_qk(m, nh, half):
                    def u():
                        xt = st["xt"]
                        if half == 0:
                            st[(m, nh)] = ps_mm.tile(
                                [128, 512], F32, tag="mm", name=f"pqk{tq}{m}{nh}"
                            )
                        pmm = st[(m, nh)]
                        for k in range(4 * half, 4 * half + 4):
                            nc.tensor.matmul(
                                pmm,
                                wqkv_sb[:, k, m * 128:(m + 1) * 128],
                                xt[:, k, nh * 512:(nh + 1) * 512],
                                start=(k == 0),
                                stop=(k == 7),
                            )
                        if half == 1:
                            dst = QT if m == 0 else KT
                            nc.vector.tensor_copy(
                                dst[:, tq * 1024 + nh * 512:
                                    tq * 1024 + (nh + 1) * 512],
                                pmm,
                            )
                    return u

                def mk_v(st_idx):
                    def u():
                        xt = st["xt"]
                        pv = ps_mm.tile([128, 128], F32, tag="mm", name=f"pv{tq}{st_idx}")
                        for k in range(8):
                            nc.tensor.matmul(
                                pv,
                                xt[:, k, st_idx * 128:(st_idx + 1) * 128],
                                wqkv_sb[:, k, 2 * HC:3 * HC],
                                start=(k == 0),
                                stop=(k == 7),
                            )
                        gt = tq * 8 + st_idx
                        b, lt = gt // 16, gt % 16
                        nc.vector.tensor_copy(Vp[:, b, lt, :, :], pv)
                    return u

                if kv_first:
                    # K and V as early as possible (attention consumes them
                    # incrementally along the kj axis); Q strips last
                    units = [u_dma]
                    for nh in range(2):
                        units += [mk_qk(1, nh, 0), mk_qk(1, nh, 1)]
                    for st_idx in range(8):
                        units.append(mk_v(st_idx))
                    for nh in range(2):
                        units += [mk_qk(0, nh, 0), mk_qk(0, nh, 1)]
                else:
                    units = [u_dma]
                    for m in range(2):
                        for nh in range(2):
                            units.append(mk_qk(m, nh, 0))
                            units.append(mk_qk(m, nh, 1))
                    for st_idx in range(8):
                        units.append(mk_v(st_idx))
                return units

            def proj_quarter_units(b):
                """Projection of this core's 256 batch-b tokens as fillers."""
                st = {}

                def u_dma():
                    agT = work.tile([128, 8, QT_TOK], BF16, tag="agT", name=f"agT{b}")
                    for r in range(NCORES):
                        nc.sync.dma_start(
                            agT[:, r, :], ag_out[b, r * HC:(r + 1) * HC, :]
                        )
                    st["agT"] = agT

                def mk_od(od, half):
                    def u():
                        agT = st["agT"]
                        if half == 0:
                            st[od] = ps_mm.tile(
                                [128, QT_TOK], F32, tag="mm", name=f"pp{b}{od}"
                            )
                        pp = st[od]
                        for r in range(4 * half, 4 * half + 4):
                            nc.tensor.matmul(
                                pp,
                                wproj_sb[:, r, od * 128:(od + 1) * 128],
                                agT[:, r, :],
                                start=(r == 0),
                                stop=(r == 7),
                            )
                        if half == 1:
                            ob = work.tile([128, QT_TOK], F32, tag="ob",
                                           name=f"ob{b}{od}")
                            nc.vector.tensor_scalar_add(
                                ob, pp, bias_sb[:, od:od + 1]
                            )
                            nc.sync.dma_start(
                                out_t[od * 128:(od + 1) * 128,
                                      b * QT_TOK:(b + 1) * QT_TOK],
                                ob,
                            )
                    return u

                return [u_dma] + [mk_od(od, hf) for od in range(8) for hf in (0, 1)]

            def a2a3_stage(half, lo, hi):
                """Stage token chunks lo..hi of the last batch's half."""
                base = 3 * N + half * 1024
                for j in range(lo, hi):
                    nc.sync.dma_start(
                        ag_in3[half, j * HC:(j + 1) * HC, :],
                        attnT[:, base + j * 128:base + (j + 1) * 128],
                    )

            def a2a3_go(half):
                nc.gpsimd.collective_compute(
                    "AllToAll", mybir.AluOpType.bypass,
                    replica_groups=[list(range(NCORES))],
                    ins=[ag_in3[half]], outs=[ag_out3[half]],
                )

            def proj_half_units(half):
                st = {}

                def u_dma():
                    agT = work.tile([128, 8, 128], BF16, tag="agT3",
                                    name=f"agT3{half}")
                    for r in range(NCORES):
                        nc.gpsimd.dma_start(
                            agT[:, r, :], ag_out3[half, r * HC:(r + 1) * HC, :]
                        )
                    st["agT"] = agT

                def mk_od(od):
                    def u():
                        agT = st["agT"]
                        pp = ps_mm.tile([128, 128], F32, tag="mm",
                                        name=f"p3{half}{od}")
                        for r in range(8):
                            nc.tensor.matmul(
                                pp, wproj_sb[:, r, od * 128:(od + 1) * 128],
                                agT[:, r, :], start=(r == 0), stop=(r == 7),
                            )
                        ob = work.tile([128, 128], F32, tag="ob",
                                       name=f"ob3{half}{od}")
                        nc.vector.tensor_scalar_add(ob, pp, bias_sb[:, od:od + 1])
                        nc.sync.dma_start(
                            out_t[od * 128:(od + 1) * 128,
                                  3 * QT_TOK + half * 128:
                                  3 * QT_TOK + half * 128 + 128],
                            ob,
                        )
                    return u

                return [u_dma] + [mk_od(od) for od in range(8)]

            # ---- minimal batch-0 prologue: x chunk 0, Q/K for kj 0-3, V 0-1
            ch0 = qkv_chunk_units(0)
            for i in (0, 1, 2, 5, 6, 9, 10):
                ch0[i]()
            ch0_rest = [ch0[i] for i in (7, 8, 11, 12, 13, 14, 15, 16, 3, 4)]
            for k in range(8):
                nc.sync.dma_start(wproj_sb[:, k, :], w_p[k * 128:(k + 1) * 128, :])
                nc.sync.dma_start(bias_sb[:, k:k + 1], b_p[k * 128:(k + 1) * 128])

            # ---- pipelined main loop over batches ----
            for b in range(B):
                t0 = b * N
                if b == 0:
                    fillers = (ch0_rest + qkv_chunk_units(1, kv_first=True)
                               + qkv_chunk_units(2) + qkv_chunk_units(3))
                else:
                    fillers = []
                    if b + 1 < B:
                        fillers += qkv_chunk_units(2 * (b + 1))
                        fillers += qkv_chunk_units(2 * (b + 1) + 1)
                    fillers += proj_quarter_units(b - 1)
                # qkv fillers are paced from iteration 0; proj fillers (which
                # wait on the previous batch's AllToAll) only from PROJ_AT on.
                n_qkv = len(fillers) - (17 if b >= 1 else 0)
                PROJ_AT = 20
                fillers.reverse()  # pop() from the end = original order
                n_fill = len(fillers)
                popped = 0

                def push_next(units):
                    """Insert units so they are popped before the backlog."""
                    nonlocal n_fill
                    fillers.extend(reversed(units))
                    n_fill += len(units)

                fin_q = []

                def pop_fin():
                    if fin_q:
                        fin_q.pop(0)()

                def finalize_units(qi, po, acc, q0):
                    """Copy out numerators, then normalize the strip without
                    touching DRAM: ones-matmul partition-reduce -> approx
                    reciprocal straight off PSUM -> K=1 matmul broadcast ->
                    one DVE multiply."""
                    st = {}

                    def u_copy():
                        nc.vector.tensor_copy(attnT[:, q0:q0 + 512], po)

                    def u_den():
                        nc.vector.tensor_add(acc[0], acc[0], acc[1])
                        pden = ps_mm.tile([2, 512], F32, tag="mm",
                                          name=f"pden{b}{qi}")
                        for h in range(HPC):
                            nc.tensor.matmul(pden, eye2[:, h, :],
                                             acc[0][:, h, :],
                                             start=(h == 0), stop=(h == 1))
                        rdenf = work.tile([2, 512], F32, tag="rdenf",
                                          name=f"rdf{b}{qi}")
                        nc.vector.reciprocal_approx_fast(rdenf, pden)
                        st["rdenf"] = rdenf

                    def u_bc():
                        rdenf = st["rdenf"]
                        bc = ps_mm.tile([128, 512], F32, tag="mm",
                                        name=f"bc{b}{qi}")
                        for h in range(HPC):
                            nc.tensor.matmul(bc[64 * h:64 * (h + 1), :],
                                             sel[:, h, :], rdenf,
                                             start=True, stop=True)
                        nc.vector.tensor_mul(
                            attnT[:, q0:q0 + 512], attnT[:, q0:q0 + 512], bc
                        )

                    return [u_copy, u_den, u_bc]

                # flat list of (qi, kj) steps; S^T pairs are emitted one step
                # ahead so the ACT-feeding matmul is never queued behind the
                # eS-gated V matmuls or filler work on the in-order PE queue
                steps = [(qi, kj) for qi in range(4) for kj in range(16)]
                pS_t = {}
                po_t = {}
                acc_t = {}

                def emit_S(qi, kj):
                    q0 = t0 + qi * 512
                    k0 = t0 + kj * 128
                    pS = ps_s.tile([128, 2, 512], F32, tag="s",
                                   name=f"pS{b}_{qi}_{kj}")
                    for h in range(HPC):
                        hs = h * HD
                        nc.tensor.matmul(
                            pS[:, h, :],
                            KT[hs:hs + HD, k0:k0 + 128],
                            QT[hs:hs + HD, q0:q0 + 512],
                            start=True,
                            stop=True,
                        )
                    pS_t[(qi, kj)] = pS

                emit_S(0, 0)
                for it, (qi, kj) in enumerate(steps):
                    q0 = t0 + qi * 512
                    if kj == 0:
                        po_t[qi] = ps_o.tile([128, 512], F32, tag="vo",
                                             name=f"po{b}_{qi}")
                        acc_t[qi] = [
                            work.tile([128, 2, 512], BF16, tag=f"acc{a}",
                                      name=f"acc{a}_{b}_{qi}")
                            for a in range(2)
                        ]
                    if b == B - 1:
                        # pipeline the last batch's own redistribute+project:
                        # half 0 overlaps strips 2-3, only strip 3's share of
                        # half 1 is left serial at the very end
                        if it == 36:
                            a2a3_stage(0, 0, 8)
                            a2a3_go(0)
                        elif it == 44:
                            push_next(proj_half_units(0))
                        elif it == 52:
                            a2a3_stage(1, 0, 4)
                    if b == 0 and it < 16:
                        target = 3 * (it + 1)
                    else:
                        target = (it + 1) * n_fill // 56
                    cap = 6 if (b == 0 and it < 16) else 4
                    quota = popped < n_qkv or it >= PROJ_AT
                    due = min(cap, max(0, target - popped)) if quota else 0
                    # sandwich the filler work around the two sem-gated
                    # instructions (S waiting its PSUM slot, V waiting eS) so
                    # the in-order PE queue never idles at a blocked head
                    pop_fin()
                    for _ in range(due // 2):
                        if fillers:
                            fillers.pop()()
                            popped += 1
                    if it + 1 < len(steps):
                        emit_S(*steps[it + 1])
                    pS = pS_t.pop((qi, kj))
                    eS = work.tile([128, 2, 512], BF16, tag="es", bufs=6)
                    nc.scalar.activation(eS, pS, EXP, scale=SCALE)
                    pop_fin()
                    for _ in range(due - due // 2):
                        if fillers:
                            fillers.pop()()
                            popped += 1
                    po, acc = po_t[qi], acc_t[qi]
                    for h in range(HPC):
                        nc.tensor.matmul(
                            po[h * HD:(h + 1) * HD, :],
                            Vp[:, b, kj, h, :],
                            eS[:, h, :],
                            start=(kj == 0),
                            stop=(kj == 15),
                        )
                    a = kj // 8
                    if kj % 8 == 0:
                        nc.vector.tensor_copy(acc[a], eS)
                    else:
                        nc.vector.tensor_add(acc[a], acc[a], eS)
                    if kj == 15:
                        fin_q += finalize_units(qi, po, acc, q0)
                while fin_q:
                    fin_q.pop(0)()
                while fillers:
                    fillers.pop()()

                if b < B - 1:
                    # ---- AllToAll for batch b ----
                    for j in range(NCORES):
                        nc.sync.dma_start(
                            ag_in[b, j * HC:(j + 1) * HC, :],
                            attnT[:, t0 + j * QT_TOK:t0 + (j + 1) * QT_TOK],
                        )
                    nc.gpsimd.collective_compute(
                        "AllToAll",
                        mybir.AluOpType.bypass,
                        replica_groups=[list(range(NCORES))],
                        ins=[ag_in[b]],
                        outs=[ag_out[b]],
                    )
                else:
                    # ---- tail: strip 3's share of the last batch ----
                    a2a3_stage(1, 4, 8)
                    a2a3_go(1)
                    for u in proj_half_units(1):
                        u()

    nc.finalize()
    return nc


def kernel(x, w_qkv, w_proj, b_proj):
    global LAST_RESULTS
    bf16 = ml_dtypes.bfloat16

    x_t = np.ascontiguousarray(x.reshape(T, DIM).T.astype(bf16))  # [DIM, T]
    w_p = np.ascontiguousarray(w_proj.astype(bf16))
    b_p = np.ascontiguousarray(b_proj.astype(np.float32))
    sel_np = np.zeros((2, 128), dtype=np.float32)
    sel_np[0, 0:64] = 1.0
    sel_np[1, 64:128] = 1.0

    in_maps = []
    for c in range(NCORES):
        w_c = np.concatenate(
            [
                w_qkv[:, HC * c:HC * (c + 1)],
                w_qkv[:, DIM + HC * c:DIM + HC * (c + 1)],
                w_qkv[:, 2 * DIM + HC * c:2 * DIM + HC * (c + 1)],
            ],
            axis=1,
        ).astype(bf16)
        in_maps.append(
            {"x_t": x_t, "w_c": np.ascontiguousarray(w_c), "w_p": w_p,
             "b_p": b_p, "sel_c": sel_np}
        )

    nc = _build()
    LAST_RESULTS = run_bass_kernel_spmd(
        nc, in_maps, core_ids=list(range(NCORES)),
        trace=bool(os.environ.get("KERNEL_TRACE")),
    )

    # core c's out_t columns: batches 0-2 are 256-token quarters; batch 3 is
    # two 128-token slices (tokens 128c and 1024+128c of the batch)
    out_T = np.empty((DIM, T), dtype=np.float32)
    for c in range(NCORES):
        res = np.asarray(LAST_RESULTS.results[c]["out_t"], dtype=np.float32)
        for b in range(B - 1):
            out_T[:, b * N + c * QT_TOK:b * N + (c + 1) * QT_TOK] = (
                res[:, b * QT_TOK:(b + 1) * QT_TOK]
            )
        b3 = (B - 1) * N
        for half in range(2):
            cols = 3 * QT_TOK + half * 128
            tok = b3 + half * 1024 + c * 128
            out_T[:, tok:tok + 128] = res[:, cols:cols + 128]
    return np.ascontiguousarray(out_T.T).reshape(B, N, DIM).astype(np.float32)
